# revision 1
# baseline (speedup 1.0000x reference)
"""BatchAll triplet loss (multi-module variant) on 8 Trainium2 NeuronCores.

Math: labels = [0..191, 0..191] -- every label appears exactly twice, so each
anchor i has exactly ONE valid positive j = (i+192) % 384.  The (i,j,k) cubic
triplet tensor therefore collapses to an (i,k) problem:

    loss_terms[i,k] = relu(d(i, p(i)) - d(i,k) + margin) * pm[i,k] * valid[i,k]
    out = sum(loss_terms) / (count(loss_terms > EPS) + EPS)

where valid excludes k in {i, p(i)} and pm = tile(weight, (2,2)).

With unit-normalized embeddings, d(i,k) = sqrt(relu(2 - 2*G[i,k]*rn_i*rn_k))
where G is the raw Gram matrix and rn = 1/||e||.  (The reference's distance
uses the normalized Gram's diagonal, which equals 1 up to 1e-7 rounding; the
constant 2 is within mutual fp32 noise.)

Weighting trick: with pmn = -pm, relu(dpos+m-d)*pm == max((d-(dpos+m))*pmn, 0)
and count(lw > EPS) == count((d-(dpos+m))*pmn > EPS) since EPS > 0.

Sharding: anchors i are blocked over the 8 cores (48 each).  Each core receives
the full embedding set TRANSPOSED and ROTATED so that its anchor slab lands at
local columns 0..47 and the positives at columns 192..239 -- one shared NEFF,
per-core data.  Each core emits its partial [sum, count]; the host reduces.

Hardware notes driving the structure (from NTFF traces):
- instructions carry at most ONE semaphore wait, so each op depends on at most
  one producer engine (Bacc legalizes violations with costly event-sem nops).
- engines execute in order: an op waiting on late data stalls everything
  behind it on that engine, so per-engine program order = readiness order.
- the PE is HAM-throttled cold (~2-4x); a few dummy matmuls during the DMA
  phase warm it before the real matmuls run.
- only sync/scalar (HWDGE) and gpsimd (SWDGE, ~6us completion latency) can
  initiate DMAs; big loads are split across the two HWDGE rings.
- a (1,384) one-lane DVE reciprocal costs 2.5us; computing 1/norm on a
  (128,3) layout and PE-transposing to rows costs ~0.5us total.
- the ACT Sqrt table load (1.3us) is pulled off the critical path by a dummy
  sqrt issued while DMAs are in flight.
"""

import os
import sys

for _p in ("/opt/trn_rl_repo", "/root/.axon_site/_ro/trn_rl_repo"):
    if _p not in sys.path:
        sys.path.append(_p)

# The SPMD dispatch path (bass2jax.run_bass_via_pjrt) takes jax.devices(), so
# the axon platform must stay visible.  If jax has not been initialized yet and
# JAX_PLATFORMS would hide it (e.g. "cpu"), clear the restriction.
if "jax" not in sys.modules and os.environ.get("JAX_PLATFORMS") in ("cpu",):
    del os.environ["JAX_PLATFORMS"]

import numpy as np

import concourse.bass as bass
import concourse.tile as tile
from concourse import mybir
from concourse.bacc import Bacc
from concourse.bass_utils import run_bass_kernel_spmd

F32 = mybir.dt.float32
ALU = mybir.AluOpType
ACT = mybir.ActivationFunctionType

B = 192          # batch (distinct labels)
N = 2 * B        # embeddings
D = 512          # embedding dim
NCORES = 8
S = N // NCORES  # anchors per core (48)
MARGIN = 0.1
EPS = 1e-8
N_WARMUP = 4     # dummy matmuls to bring the PE out of HAM throttle


def build_nc() -> bass.Bass:
    nc = Bacc()

    embt = nc.dram_tensor("embt", [D, N], F32, kind="ExternalInput")
    embr = nc.dram_tensor("embr", [N, D], F32, kind="ExternalInput")
    pmwn = nc.dram_tensor("pmwn", [S, N], F32, kind="ExternalInput")
    out = nc.dram_tensor("out", [1, 2], F32, kind="ExternalOutput")

    KC = D // 128   # contraction chunks for the Gram (4)
    RT = N // 128   # row-layout tiles / rn row chunks (3)

    with tile.TileContext(nc) as tc:
        with (
            tc.tile_pool(name="sb", bufs=1) as sb,
            tc.tile_pool(name="ps", bufs=1, space="PSUM") as ps,
        ):
            E, R = [], []
            for c in range(KC):
                e = sb.tile([128, N], F32, tag=f"E{c}")
                E.append(e)
            for t in range(RT):
                r = sb.tile([128, D], F32, tag=f"R{t}")
                R.append(r)
            pm = sb.tile([S, N], F32, tag="pm")

            # ---- loads: E chunks first so the Gram follows the PE warm-up
            #      with no idle gap (HAM re-throttles after ~1us idle) ----
            nc.sync.dma_start(out=E[0], in_=embt[0:128, :])
            nc.scalar.dma_start(out=E[2], in_=embt[256:384, :])
            nc.sync.dma_start(out=R[0], in_=embr[0:128, :])
            nc.scalar.dma_start(out=R[1], in_=embr[128:256, :])
            nc.sync.dma_start(out=E[1], in_=embt[128:256, :])
            nc.scalar.dma_start(out=E[3], in_=embt[384:512, :])
            nc.sync.dma_start(out=R[2], in_=embr[256:384, :])
            nc.gpsimd.dma_start(out=pm, in_=pmwn[:, :])   # needed late; SWDGE ok

            # ---- identity via iota on the (otherwise idle) gpsimd ----
            icol = sb.tile([128, 128], F32, tag="icol")
            nc.gpsimd.iota(icol, [[1, 128]], channel_multiplier=0,
                           allow_small_or_imprecise_dtypes=True)
            iprt = sb.tile([128, 1], F32, tag="iprt")
            nc.gpsimd.iota(iprt, [[0, 1]], channel_multiplier=1,
                           allow_small_or_imprecise_dtypes=True)
            ident = sb.tile([128, 128], F32, tag="ident")
            nc.gpsimd.tensor_scalar(ident, icol, iprt, None, op0=ALU.is_equal)

            # ---- warm-up scaffolding ----
            wtile = sb.tile([128, 256], F32, tag="wtile")
            nc.vector.memset(wtile, 1.0)
            ones_col = sb.tile([S, 1], F32, tag="ones_col")
            nc.vector.memset(ones_col, 1.0)
            ones_row = sb.tile([1, S], F32, tag="ones_row")
            nc.vector.memset(ones_row, 1.0)
            twos_col = sb.tile([S, 1], F32, tag="twos_col")
            nc.vector.memset(twos_col, 2.0)
            tdum = sb.tile([1, 1], F32, tag="tdum")
            nc.scalar.sqrt(tdum, wtile[0:1, 0:1])   # pull ACT sqrt table early

            wps = ps.tile([128, 256], F32, tag="wps")
            for _ in range(N_WARMUP):
                nc.tensor.matmul(wps, wtile[:, 0:128], wtile,
                                 start=True, stop=True)

            # ---- norms on DVE: ns_col[p,t] = ||emb[128t+p]||^2 ----
            ns_col = sb.tile([128, RT], F32, tag="ns_col")
            junk = sb.tile([128, D], F32, tag="junk")
            for t in range(RT):
                nc.vector.scalar_tensor_tensor(
                    junk, R[t], 1.0, R[t], op0=ALU.mult, op1=ALU.mult,
                    accum_out=ns_col[:, t:t + 1])
            nrm_col = sb.tile([128, RT], F32, tag="nrm_col")
            nc.scalar.sqrt(nrm_col, ns_col)
            rn_col = sb.tile([128, RT], F32, tag="rn_col")   # 1/||e||
            nc.vector.reciprocal(rn_col, nrm_col)

            # ---- Gram slab: G[a,k] = sum_d embt[d,a] * embt[d,k] ----
            g_ps = ps.tile([S, N], F32, tag="G")
            for c in range(KC):
                nc.tensor.matmul(g_ps, E[c][:, 0:S], E[c],
                                 start=(c == 0), stop=(c == KC - 1))

            # ---- -2*rn_a row scale of G (DVE work while PE transposes) ----
            rnam2 = sb.tile([S, 1], F32, tag="rnam2")        # -2 * rn[slab]
            nc.vector.tensor_scalar_mul(rnam2, rn_col[0:S, 0:1], -2.0)
            x1 = sb.tile([S, N], F32, tag="X1")              # -2 rn_a G
            nc.vector.tensor_scalar_mul(x1, g_ps, rnam2)

            # ---- rn to rows via PE transposes; copies split DVE/ACT ----
            rn_t = []
            for j in range(RT):
                rj_ps = ps.tile([1, 128], F32, tag=f"rnT{j}")
                nc.tensor.transpose(rj_ps, rn_col[:, j:j + 1], ident)
                rj = sb.tile([1, 128], F32, tag=f"rn_t{j}")
                if j == 1:
                    nc.scalar.copy(rj, rj_ps)      # gpsimd can't read PSUM
                else:
                    nc.vector.tensor_copy(rj, rj_ps)
                rn_t.append(rj)

            # ---- RB[a,k] = rn_k broadcast (rank-1, 128 cols per chunk) ----
            rb_ps = ps.tile([S, N], F32, tag="RB")
            for j in range(RT):
                nc.tensor.matmul(rb_ps[:, j * 128:(j + 1) * 128], ones_row,
                                 rn_t[j], start=True, stop=True)

            # ---- d2 = relu(2 - 2 * G * rn_a * rn_k) ----
            t1 = sb.tile([S, N], F32, tag="T1")              # -2 rn_a rn_k G
            nc.vector.tensor_mul(t1, x1, rb_ps)
            d2 = sb.tile([S, N], F32, tag="D2")
            nc.vector.tensor_scalar(d2, t1, 2.0, 0.0, op0=ALU.add, op1=ALU.max)
            dms = sb.tile([S, N], F32, tag="dms")
            nc.scalar.sqrt(dms, d2)

            # ---- positive distance straight from t1's diagonal block
            #      (pre-relu; d2_pos ~ 2 > 0 always): dpos = sqrt(t1_pos + 2),
            #      with the +2 folded into the sqrt bias ----
            dpb = sb.tile([S, S], F32, tag="dpb")
            t1pos = sb.tile([S, 1], F32, tag="t1pos")
            nc.vector.scalar_tensor_tensor(
                dpb, t1[:, B:B + S], 1.0, ident[0:S, 0:S], op0=ALU.mult,
                op1=ALU.mult, accum_out=t1pos)
            dpos = sb.tile([S, 1], F32, tag="dpos")
            nc.scalar.activation(dpos, t1pos, ACT.Sqrt, bias=twos_col, scale=1.0)
            dpos_m = sb.tile([S, 1], F32, tag="dpos_m")
            nc.vector.tensor_scalar_add(dpos_m, dpos, MARGIN)

            # ---- weighted triplet terms via the negated-weight trick ----
            lwpre = sb.tile([S, N], F32, tag="lwpre")
            nc.vector.scalar_tensor_tensor(
                lwpre, dms, dpos_m, pm, op0=ALU.subtract, op1=ALU.mult)
            stacked = sb.tile([S, 2], F32, tag="stacked")
            lw = sb.tile([S, N], F32, tag="LW")
            nc.vector.tensor_scalar(
                lw, lwpre, 0.0, 0.0, op0=ALU.max, op1=ALU.add,
                accum_out=stacked[:, 0:1])
            c01 = sb.tile([S, N], F32, tag="C01")
            nc.vector.tensor_scalar(
                c01, lwpre, EPS, 0.0, op0=ALU.is_gt, op1=ALU.add,
                accum_out=stacked[:, 1:2])

            # ---- cross-partition reduce: out[0,:] = sum_a stacked[a,:] ----
            out_ps = ps.tile([1, 2], F32, tag="out")
            nc.tensor.matmul(out_ps, ones_col, stacked, start=True, stop=True)
            outs = sb.tile([1, 2], F32, tag="outs")
            nc.scalar.copy(outs, out_ps)
            nc.sync.dma_start(out=out[:, :], in_=outs)

    nc.finalize()
    return nc


_NC_CACHE: dict = {}


def _get_nc() -> bass.Bass:
    if "nc" not in _NC_CACHE:
        _NC_CACHE["nc"] = build_nc()
    return _NC_CACHE["nc"]


def make_in_maps(output1, output2, weight):
    o1 = np.asarray(output1, dtype=np.float32)
    o2 = np.asarray(output2, dtype=np.float32)
    w = np.asarray(weight, dtype=np.float32)

    emb = np.concatenate([o1, o2], axis=0)  # (384, 512) unnormalized
    aS = np.arange(S)

    in_maps = []
    for c in range(NCORES):
        rot = (np.arange(N) + c * S) % N                  # local -> global
        embr = np.ascontiguousarray(emb[rot])             # (384, 512)
        embt = np.ascontiguousarray(embr.T)               # (512, 384)
        pmw = np.ascontiguousarray(w[rot[:S] % B][:, rot % B])  # (48, 384)
        pmw[aS, aS] = 0.0          # k == i
        pmw[aS, B + aS] = 0.0      # k == p(i)
        in_maps.append({"embt": embt, "embr": embr, "pmwn": -pmw})
    return in_maps


def reduce_outputs(results):
    parts = np.stack([r["out"][0] for r in results])      # (8, 2)
    total = parts.sum(axis=0, dtype=np.float32)
    return np.asarray(
        np.float32(total[0]) / (np.float32(total[1]) + np.float32(EPS)),
        dtype=np.float32)


def kernel(output1, output2, weight):
    in_maps = make_in_maps(output1, output2, weight)
    res = run_bass_kernel_spmd(_get_nc(), in_maps, core_ids=list(range(NCORES)))
    return reduce_outputs(res.results)



# revision 4
# speedup vs baseline: 1.1442x; 1.1442x over previous
"""BatchAll triplet loss (multi-module variant) on 8 Trainium2 NeuronCores.

Math: labels = [0..191, 0..191] -- every label appears exactly twice, so each
anchor i has exactly ONE valid positive j = (i+192) % 384.  The (i,j,k) cubic
triplet tensor collapses to an (i,k) problem:

    loss_terms[i,k] = relu(d(i, p(i)) - d(i,k) + margin) * pm[i,k] * valid[i,k]
    out = sum(loss_terms) / (count(loss_terms > EPS) + EPS)

With unit-normalized embeddings, d(i,k) = sqrt(2 + delta - 2*G[i,k]*rn_i*rn_k)
where G is the raw Gram matrix and rn = 1/||e||; delta=1e-5 keeps the masked
diagonal (cos=1) non-negative so no separate relu clamp is needed (the +delta
shifts every distance by <4e-6 -- far below the 2e-2 gate).

Precision: embeddings ship as fp8_e4m3 (Gram on the PE in fp8, norms from the
SAME fp8 values so the diagonal cancels exactly); weights ship as bf16.
Measured end-to-end rel-err of this scheme vs the fp32 reference: ~1e-3.

Layout: [128, 192] -- partitions 0:48 anchors x k-block 0 (local k 0..191),
partitions 64:112 anchors x k-block 1 (local k 192..383); partitions 48:64 and
112:128 are pad (driven by 16 junk lhsT columns, masked by pm=0).  Local column
order per core: [anchors | positives | rest], so the positive distances sit on
the diagonal of t2[0:48, 48:96] and self-pairs on diag of t2[0:48, 0:48].

The weighted-term trick: with pmn = -pm, relu(dpos+m-d)*pm == max((d-dposm)*pmn, 0)
and count(term > EPS) == count((d-dposm)*pmn > EPS).  count is computed as
sum(sign((d-dposm)*pmn - EPS)) on the ACT engine (in parallel with the DVE
sum-reduce); the host maps sign-sum -> count via count = (ssum + cells)/2.

Sharding: anchors blocked over 8 cores (48 each); host reduces [sum, signsum].
"""

import os
import sys

for _p in ("/opt/trn_rl_repo", "/root/.axon_site/_ro/trn_rl_repo"):
    if _p not in sys.path:
        sys.path.append(_p)

if "jax" not in sys.modules and os.environ.get("JAX_PLATFORMS") in ("cpu",):
    del os.environ["JAX_PLATFORMS"]

import ml_dtypes
import numpy as np

import concourse.bass as bass
import concourse.tile as tile
from concourse import mybir
from concourse.bacc import Bacc
from concourse.bass_utils import run_bass_kernel_spmd

F32 = mybir.dt.float32
BF16 = mybir.dt.bfloat16
F8 = mybir.dt.float8e4
ALU = mybir.AluOpType
ACT = mybir.ActivationFunctionType

B = 192          # batch (distinct labels)
N = 2 * B        # embeddings
D = 512          # embedding dim
NCORES = 8
S = N // NCORES  # anchors per core (48)
MARGIN = 0.1
EPS = 1e-8
DELTA = 1e-5     # d2 positivity bias
CELLS = 128 * 192 * NCORES  # sign-sum -> count affine constant
N_WARMUP = 8     # fp8 warmup matmuls bridging the DMA phase (HAM throttle)


def build_nc() -> bass.Bass:
    nc = Bacc()

    et0 = nc.dram_tensor("et0", [128, 768], F8, kind="ExternalInput")
    et1 = nc.dram_tensor("et1", [128, 768], F8, kind="ExternalInput")
    er0 = nc.dram_tensor("er0", [128, 1024], F8, kind="ExternalInput")
    er1 = nc.dram_tensor("er1", [128, 512], F8, kind="ExternalInput")
    pmw = nc.dram_tensor("pmw", [128, 192], BF16, kind="ExternalInput")
    out = nc.dram_tensor("out", [1, 2], F32, kind="ExternalOutput")

    with tile.TileContext(nc) as tc:
        with (
            tc.tile_pool(name="sb", bufs=1) as sb,
            tc.tile_pool(name="ps", bufs=1, space="PSUM") as ps,
        ):
            ET = sb.tile([128, 1536], F8, tag="ET")    # embt packed: chunk c at cols 384c
            ER = sb.tile([128, 1536], F8, tag="ER")    # embr packed: row-chunk t at cols 512t
            pm = sb.tile([128, 192], BF16, tag="pm")   # negated masked weights

            # ---- input DMAs: all four embedding loads on the sync HWDGE ring
            #      (one queue saturates the DMA engines; issue order = need
            #      order).  pm goes on the scalar ring after the ACT table
            #      load; it's needed last. ----
            nc.sync.dma_start(out=ER[:, 0:1024], in_=er0[:, :])
            nc.sync.dma_start(out=ER[:, 1024:1536], in_=er1[:, :])
            nc.sync.dma_start(out=ET[:, 0:768], in_=et0[:, :])
            nc.sync.dma_start(out=ET[:, 768:1536], in_=et1[:, :])

            # ---- identity via iota on gpsimd (needed ~t+3us; is_equal is slow) ----
            icol = sb.tile([128, 128], F32, tag="icol")
            nc.gpsimd.iota(icol, [[1, 128]], channel_multiplier=0,
                           allow_small_or_imprecise_dtypes=True)
            iprt = sb.tile([128, 1], F32, tag="iprt")
            nc.gpsimd.iota(iprt, [[0, 1]], channel_multiplier=1,
                           allow_small_or_imprecise_dtypes=True)
            ident = sb.tile([128, 128], F32, tag="ident")
            nc.gpsimd.tensor_scalar(ident, icol, iprt, None, op0=ALU.is_equal)

            # ---- DVE constants ----
            wtile = sb.tile([128, 512], F8, tag="wtile")
            nc.vector.memset(wtile, 1.0)
            ones1 = sb.tile([1, 64], F32, tag="ones1")     # rank-1 broadcast lhsT
            nc.vector.memset(ones1, 1.0)
            onesc = sb.tile([128, 1], F32, tag="onesc")    # final reduce lhsT
            nc.vector.memset(onesc, 1.0)
            b2 = sb.tile([128, 1], F32, tag="b2")          # sqrt bias 2+delta
            nc.vector.memset(b2, 2.0 + DELTA)
            beps = sb.tile([128, 1], F32, tag="beps")      # sign bias -EPS
            nc.vector.memset(beps, -EPS)

            # ---- scalar engine: dummy sqrt pulls the ACT table early, then
            #      the pm DMA rides the scalar HWDGE ring ----
            tdum = sb.tile([1, 1], F32, tag="tdum")
            nc.scalar.sqrt(tdum, b2[0:1, 0:1])
            nc.scalar.dma_start(out=pm, in_=pmw[:, :])

            # ---- PE warm-up (HAM throttle) while DMAs are in flight ----
            wps = ps.tile([128, 256], F32, tag="wps")
            for _ in range(N_WARMUP):
                nc.tensor.matmul(wps, wtile[:, 0:128], wtile[:, 0:256],
                                 start=True, stop=True)

            # ---- norms from the SAME fp8 values as the Gram:
            #      ns_col[p,t] = ||emb_loc[128t+p]||^2; chunks split DVE/ACT ----
            ns_col = sb.tile([128, 3], F32, tag="ns_col")
            junk = sb.tile([128, 512], F32, tag="junk")
            junk2 = sb.tile([128, 512], F32, tag="junk2")
            nc.vector.scalar_tensor_tensor(
                junk, ER[:, 0:512], 1.0, ER[:, 0:512], op0=ALU.mult,
                op1=ALU.mult, accum_out=ns_col[:, 0:1])
            nc.scalar.activation(junk2, ER[:, 512:1024], ACT.Square,
                                 accum_out=ns_col[:, 1:2])
            nc.vector.scalar_tensor_tensor(
                junk, ER[:, 1024:1536], 1.0, ER[:, 1024:1536], op0=ALU.mult,
                op1=ALU.mult, accum_out=ns_col[:, 2:3])

            # ---- Gram slab in [128,192] layout: 2 column-blocks x 4 chunks;
            #      lhsT = 64 local cols (48 anchors + 16 pad) ----
            g_ps = ps.tile([128, 192], F32, tag="G")
            for c in range(4):
                lhsT = ET[:, 384 * c:384 * c + 64]
                nc.tensor.matmul(g_ps[0:64, :], lhsT,
                                 ET[:, 384 * c:384 * c + 192],
                                 start=(c == 0), stop=(c == 3))
                nc.tensor.matmul(g_ps[64:128, :], lhsT,
                                 ET[:, 384 * c + 192:384 * c + 384],
                                 start=(c == 0), stop=(c == 3))

            # ---- rn = 1/||e||: sqrt (ACT) + reciprocal (DVE) on [128,3] ----
            nrm = sb.tile([128, 3], F32, tag="nrm")
            nc.scalar.sqrt(nrm, ns_col)
            rn_col = sb.tile([128, 3], F32, tag="rn_col")
            nc.vector.reciprocal(rn_col, nrm)

            # ---- sel2[c,p] = 1 iff p%64==c (c<48): the partition duplicator ----
            sel2 = sb.tile([48, 128], F32, tag="sel2")
            nc.vector.memset(sel2, 0.0)
            nc.vector.tensor_copy(sel2[:, 0:48], ident[0:48, 0:48])
            nc.vector.tensor_copy(sel2[:, 64:112], ident[0:48, 0:48])

            # ---- rn_a per partition: rnA[p] = rn[p%64] (pad cols give 0) ----
            rnA_ps = ps.tile([128, 1], F32, tag="rnA")
            nc.tensor.matmul(rnA_ps, sel2, rn_col[0:48, 0:1],
                             start=True, stop=True)

            # ---- rn to rows via PE transposes (one PSUM bank, 3 col slices) ----
            rts_ps = ps.tile([1, 384], F32, tag="rnT")
            for j in range(3):
                nc.tensor.transpose(rts_ps[0:1, 128 * j:128 * (j + 1)],
                                    rn_col[:, j:j + 1], ident)

            # ---- RB[p,f] = rn_loc[192*(p//64) + f] via 4 rank-1 panels ----
            r0 = sb.tile([1, 128], F32, tag="r0")
            r1 = sb.tile([1, 128], F32, tag="r1")
            r2 = sb.tile([1, 128], F32, tag="r2")
            nc.vector.tensor_copy(r0, rts_ps[0:1, 0:128])
            nc.vector.tensor_copy(r1, rts_ps[0:1, 128:256])
            nc.vector.tensor_copy(r2, rts_ps[0:1, 256:384])
            rb_ps = ps.tile([128, 192], F32, tag="RB")
            nc.tensor.matmul(rb_ps[0:64, 0:128], ones1, r0,
                             start=True, stop=True)
            nc.tensor.matmul(rb_ps[0:64, 128:192], ones1, r1[0:1, 0:64],
                             start=True, stop=True)
            nc.tensor.matmul(rb_ps[64:128, 0:64], ones1, r1[0:1, 64:128],
                             start=True, stop=True)
            nc.tensor.matmul(rb_ps[64:128, 64:192], ones1, r2,
                             start=True, stop=True)

            # ---- t2 = -2 * G * rn_a * rn_k ----
            rnm2 = sb.tile([128, 1], F32, tag="rnm2")
            nc.scalar.mul(rnm2, rnA_ps, -2.0)
            x1 = sb.tile([128, 192], F32, tag="x1")
            nc.vector.tensor_scalar_mul(x1, g_ps, rnm2)
            t2_ps = ps.tile([128, 192], F32, tag="t2")
            nc.vector.tensor_mul(t2_ps, x1, rb_ps)

            # ---- distances: d = sqrt(t2 + 2 + delta) ----
            dms = sb.tile([128, 192], F32, tag="dms")
            nc.scalar.activation(dms, t2_ps, ACT.Sqrt, bias=b2, scale=1.0)

            # ---- positive distances from diag of t2[0:48, 48:96] ----
            tpj = sb.tile([48, 48], F32, tag="tpj")
            t2pos = sb.tile([48, 1], F32, tag="t2pos")
            nc.vector.scalar_tensor_tensor(
                tpj, t2_ps[0:48, 48:96], 1.0, ident[0:48, 0:48], op0=ALU.mult,
                op1=ALU.mult, accum_out=t2pos)
            tp_ps = ps.tile([128, 1], F32, tag="tp")
            nc.tensor.matmul(tp_ps, sel2, t2pos, start=True, stop=True)
            dpos = sb.tile([128, 1], F32, tag="dpos")
            nc.scalar.activation(dpos, tp_ps, ACT.Sqrt, bias=b2, scale=1.0)
            dposm = sb.tile([128, 1], F32, tag="dposm")
            nc.vector.tensor_scalar_add(dposm, dpos, MARGIN)

            # ---- weighted terms; sum on DVE, sign-count on ACT (parallel) ----
            lwpre = sb.tile([128, 192], F32, tag="lwpre")
            nc.vector.scalar_tensor_tensor(
                lwpre, dms, dposm, pm, op0=ALU.subtract, op1=ALU.mult)
            stacked = sb.tile([128, 2], F32, tag="stacked")
            lwj = sb.tile([128, 192], F32, tag="lwj")
            nc.vector.tensor_scalar(
                lwj, lwpre, 0.0, 0.0, op0=ALU.max, op1=ALU.add,
                accum_out=stacked[:, 0:1])
            sgj = sb.tile([128, 192], F32, tag="sgj")
            nc.scalar.activation(sgj, lwpre, ACT.Sign, bias=beps, scale=1.0,
                                 accum_out=stacked[:, 1:2])

            # ---- cross-partition reduce + writeback ----
            outp = ps.tile([1, 2], F32, tag="outp")
            nc.tensor.matmul(outp, onesc, stacked, start=True, stop=True)
            outs = sb.tile([1, 2], F32, tag="outs")
            nc.scalar.copy(outs, outp)
            nc.sync.dma_start(out=out[:, :], in_=outs)

    nc.finalize()
    return nc


_NC_CACHE: dict = {}


def _get_nc() -> bass.Bass:
    if "nc" not in _NC_CACHE:
        _NC_CACHE["nc"] = build_nc()
    return _NC_CACHE["nc"]


def make_in_maps(output1, output2, weight):
    o1 = np.asarray(output1, dtype=np.float32)
    o2 = np.asarray(output2, dtype=np.float32)
    w = np.asarray(weight, dtype=np.float32)

    emb = np.concatenate([o1, o2], axis=0)          # (384, 512) unnormalized
    w2 = np.tile(w, (2, 2))                          # (384, 384)
    f8 = ml_dtypes.float8_e4m3
    a48 = np.arange(S)

    in_maps = []
    for c in range(NCORES):
        anchors = np.arange(c * S, c * S + S)
        pos = (anchors + B) % N
        used = np.zeros(N, dtype=bool)
        used[anchors] = True
        used[pos] = True
        loc = np.concatenate([anchors, pos, np.nonzero(~used)[0]])

        emb_loc = np.ascontiguousarray(emb[loc])     # (384, 512)
        embt = emb_loc.T                             # (512, 384)
        ET = np.concatenate([embt[128 * k:128 * (k + 1), :] for k in range(4)],
                            axis=1)                  # (128, 1536)
        ER = np.concatenate([emb_loc[128 * t:128 * (t + 1), :] for t in range(3)],
                            axis=1)                  # (128, 1536)
        ET = ET.astype(f8)
        ER = ER.astype(f8)

        pmn = np.zeros((128, 192), dtype=np.float32)
        pmn[0:48, :] = -w2[anchors[:, None], loc[None, 0:192]]
        pmn[64:112, :] = -w2[anchors[:, None], loc[None, 192:384]]
        pmn[a48, a48] = 0.0          # k == i       (local col a, block 0)
        pmn[a48, S + a48] = 0.0      # k == p(i)    (local col 48+a, block 0)

        in_maps.append({
            "et0": np.ascontiguousarray(ET[:, 0:768]),
            "et1": np.ascontiguousarray(ET[:, 768:1536]),
            "er0": np.ascontiguousarray(ER[:, 0:1024]),
            "er1": np.ascontiguousarray(ER[:, 1024:1536]),
            "pmw": pmn.astype(ml_dtypes.bfloat16),
        })
    return in_maps


def reduce_outputs(results):
    parts = np.stack([np.asarray(r["out"][0], dtype=np.float64)
                      for r in results])            # (8, 2)
    total = parts.sum(axis=0)
    count = (total[1] + CELLS) / 2.0
    return np.asarray(
        np.float32(total[0]) / (np.float32(count) + np.float32(EPS)),
        dtype=np.float32)


def kernel(output1, output2, weight):
    in_maps = make_in_maps(output1, output2, weight)
    res = run_bass_kernel_spmd(_get_nc(), in_maps, core_ids=list(range(NCORES)))
    return reduce_outputs(res.results)


# revision 7
# speedup vs baseline: 1.2575x; 1.0990x over previous
"""BatchAll triplet loss (multi-module variant) on 8 Trainium2 NeuronCores.

Math: labels = [0..191, 0..191] -- each anchor i has exactly ONE valid positive
j = (i+192) % 384, so the (i,j,k) cubic triplet tensor collapses to (i,k):

    loss_terms[i,k] = relu(d(i, p(i)) - d(i,k) + margin) * pm[i,k] * valid[i,k]
    out = sum(loss_terms) / (count(loss_terms > EPS) + EPS)

d(i,k) = sqrt(relu(2 + delta - 2*G[i,k]*rn_i*rn_k)) with raw Gram G and
rn = 1/||e||; the explicit relu guards the masked diagonal against the bf16
rounding of rn (delta=1e-5 keeps exact-diagonal sqrt well-defined).

Precision: embeddings as fp8_e4m3 (PE Gram in fp8; norms from the SAME fp8
values so the diagonal cancels), rn/selector/broadcast matmuls in bf16
(single-pass PE), weights bf16.  Measured rel-err vs fp32 reference ~1e-4.

Layout: [128, 192] -- partitions 0:48 anchors x k-block 0 (local k 0..191),
64:112 anchors x k-block 1; 48:64 and 112:128 are pad driven by junk lhsT
columns and masked by pm=0.  Local column order: [anchors | positives | rest],
so positives sit on diag of t2[0:48, 48:96].

dpos rides as column 192 of a [128,193] tile so one ACT sqrt produces the
whole distance grid AND the positive distances.  count is computed as
sum(sign(lwpre - EPS)) on ACT (parallel with the DVE sum-reduce); the host
maps sign-sum -> count = (ssum + cells)/2.
"""

import os
import sys

for _p in ("/opt/trn_rl_repo", "/root/.axon_site/_ro/trn_rl_repo"):
    if _p not in sys.path:
        sys.path.append(_p)

if "jax" not in sys.modules and os.environ.get("JAX_PLATFORMS") in ("cpu",):
    del os.environ["JAX_PLATFORMS"]

import ml_dtypes
import numpy as np

import concourse.bass as bass
import concourse.tile as tile
from concourse import mybir
from concourse.bacc import Bacc
from concourse.bass_utils import run_bass_kernel_spmd

F32 = mybir.dt.float32
BF16 = mybir.dt.bfloat16
F8 = mybir.dt.float8e4
ALU = mybir.AluOpType
ACT = mybir.ActivationFunctionType

B = 192
N = 2 * B
D = 512
NCORES = 8
S = N // NCORES          # 48 anchors per core
MARGIN = 0.1
EPS = 1e-8
DELTA = 1e-5
CELLS = 128 * 192 * NCORES
N_WARMUP = 12


def build_nc() -> bass.Bass:
    nc = Bacc()

    er = nc.dram_tensor("er", [128, 1536], F8, kind="ExternalInput")
    et0 = nc.dram_tensor("et0", [128, 768], F8, kind="ExternalInput")
    et1 = nc.dram_tensor("et1", [128, 768], F8, kind="ExternalInput")
    pmw = nc.dram_tensor("pmw", [128, 192], BF16, kind="ExternalInput")
    cst = nc.dram_tensor("cst", [128, 448], BF16, kind="ExternalInput")
    out = nc.dram_tensor("out", [1, 2], F32, kind="ExternalOutput")

    with tile.TileContext(nc) as tc:
        with (
            tc.tile_pool(name="sb", bufs=1) as sb,
            tc.tile_pool(name="ps", bufs=1, space="PSUM") as ps,
        ):
            ET = sb.tile([128, 1536], F8, tag="ET")
            ER = sb.tile([128, 1536], F8, tag="ER")
            pm = sb.tile([128, 192], BF16, tag="pm")
            cs = sb.tile([128, 448], BF16, tag="cs")
            identB = cs[:, 0:128]          # identity
            sel2 = cs[0:48, 128:256]       # sel2[c,p]=1 iff p%64==c
            sel2m = cs[0:48, 256:384]      # -2 * sel2
            ones1 = cs[0:1, 384:448]       # row of 64 ones

            # ---- DMAs: embeddings on the sync ring (ER first: it heads the
            #      longest chain), consts on gpsimd SWDGE, pm on the scalar
            #      ring sandwiched between the ACT table loads ----
            nc.sync.dma_start(out=ER, in_=er[:, :])
            nc.sync.dma_start(out=ET[:, 0:768], in_=et0[:, :])
            nc.sync.dma_start(out=ET[:, 768:1536], in_=et1[:, :])
            nc.gpsimd.dma_start(out=cs, in_=cst[:, :])

            # ---- DVE constants ----
            wtile = sb.tile([128, 256], F8, tag="wtile")
            nc.vector.memset(wtile, 1.0)
            onesc = sb.tile([128, 1], F32, tag="onesc")
            nc.vector.memset(onesc, 1.0)
            beps = sb.tile([128, 1], F32, tag="beps")
            nc.vector.memset(beps, -EPS)

            # ---- scalar engine: dummy sqrt pulls the ACT table early ----
            tdum = sb.tile([1, 1], F32, tag="tdum")
            nc.scalar.sqrt(tdum, onesc[0:1, 0:1])
            nc.scalar.dma_start(out=pm, in_=pmw[:, :])

            # ---- PE warm-up bridging the DMA phase ----
            wps = ps.tile([128, 256], F32, tag="wps")
            for _ in range(N_WARMUP):
                nc.tensor.matmul(wps, wtile[:, 0:128], wtile,
                                 start=True, stop=True)

            # ---- norms from the fp8 rows, split DVE/ACT ----
            ns_col = sb.tile([128, 4], F32, tag="ns_col")
            junk = sb.tile([128, 512], BF16, tag="junk")
            junk2 = sb.tile([128, 512], BF16, tag="junk2")
            nc.vector.scalar_tensor_tensor(
                junk, ER[:, 0:512], 1.0, ER[:, 0:512], op0=ALU.mult,
                op1=ALU.mult, accum_out=ns_col[:, 0:1])
            nc.scalar.activation(junk2[:, 0:512], ER[:, 512:1024], ACT.Square,
                                 accum_out=ns_col[:, 1:2])
            nc.vector.scalar_tensor_tensor(
                junk[:, 0:256], ER[:, 1280:1536], 1.0, ER[:, 1280:1536],
                op0=ALU.mult, op1=ALU.mult, accum_out=ns_col[:, 3:4])
            nc.scalar.activation(junk2[:, 0:256], ER[:, 1024:1280], ACT.Square,
                                 accum_out=ns_col[:, 2:3])
            nc.vector.tensor_tensor(ns_col[:, 2:3], ns_col[:, 2:3],
                                    ns_col[:, 3:4], op=ALU.add)

            # ---- Gram in [128,192] layout: 2 blocks x 4 chunks, fp8 ----
            g_ps = ps.tile([128, 192], F32, tag="G")
            for c in range(4):
                lhsT = ET[:, 384 * c:384 * c + 64]
                nc.tensor.matmul(g_ps[0:64, :], lhsT,
                                 ET[:, 384 * c:384 * c + 192],
                                 start=(c == 0), stop=(c == 3))
                nc.tensor.matmul(g_ps[64:128, :], lhsT,
                                 ET[:, 384 * c + 192:384 * c + 384],
                                 start=(c == 0), stop=(c == 3))

            # ---- rn = 1/sqrt(ns) in bf16 (bf16 rounding is guarded by the
            #      explicit relu on d2) ----
            nrm = sb.tile([128, 3], F32, tag="nrm")
            nc.scalar.sqrt(nrm, ns_col[:, 0:3])
            rn_col = sb.tile([128, 3], BF16, tag="rn_col")
            with nc.allow_low_precision("bf16 rn; relu-guarded downstream"):
                nc.vector.reciprocal(rn_col, nrm)

            # ---- rnA[p] = -2*rn[p%64] via selector matmul (bf16, 1-pass) ----
            rnA_ps = ps.tile([128, 1], F32, tag="rnA")
            nc.tensor.matmul(rnA_ps, sel2m, rn_col[0:48, 0:1],
                             start=True, stop=True)

            # ---- rn to one partition-0 row [1,384] via 3 transposes ----
            rts_ps = ps.tile([1, 384], BF16, tag="rnT")
            for j in range(3):
                nc.tensor.transpose(rts_ps[0:1, 128 * j:128 * (j + 1)],
                                    rn_col[:, j:j + 1], identB)
            rrow = sb.tile([1, 384], BF16, tag="rrow")
            with nc.allow_low_precision("bf16 rn rows; relu-guarded"):
                nc.vector.tensor_copy(rrow, rts_ps)

            # ---- RB[p,f] = rn_loc[192*(p//64) + f] via 4 rank-1 panels ----
            rb_ps = ps.tile([128, 192], F32, tag="RB")
            nc.tensor.matmul(rb_ps[0:64, 0:128], ones1, rrow[0:1, 0:128],
                             start=True, stop=True)
            nc.tensor.matmul(rb_ps[0:64, 128:192], ones1, rrow[0:1, 128:192],
                             start=True, stop=True)
            nc.tensor.matmul(rb_ps[64:128, 0:64], ones1, rrow[0:1, 192:256],
                             start=True, stop=True)
            nc.tensor.matmul(rb_ps[64:128, 64:192], ones1, rrow[0:1, 256:384],
                             start=True, stop=True)

            # ---- t2 = -2 * G * rn_a * rn_k ----
            rnAsb = sb.tile([128, 1], F32, tag="rnAsb")
            nc.vector.tensor_copy(rnAsb, rnA_ps)
            x1 = sb.tile([128, 192], F32, tag="x1")
            nc.vector.tensor_scalar_mul(x1, g_ps, rnAsb)
            t2_ps = ps.tile([128, 192], F32, tag="t2")
            nc.vector.tensor_mul(t2_ps, x1, rb_ps)

            # ---- positive-pair t2 values -> bf16 -> duplicated [128,1] ----
            tpj = sb.tile([48, 48], F32, tag="tpj")
            t2pos = sb.tile([48, 1], BF16, tag="t2pos")
            with nc.allow_low_precision("bf16 dpos path; |err| ~3e-3 abs"):
                nc.vector.scalar_tensor_tensor(
                    tpj, t2_ps[0:48, 48:96], 1.0, identB[0:48, 0:48],
                    op0=ALU.mult, op1=ALU.mult, accum_out=t2pos)
            tp_ps = ps.tile([128, 1], F32, tag="tp")
            nc.tensor.matmul(tp_ps, sel2, t2pos, start=True, stop=True)

            # ---- d2 (grid in cols 0:192, dpos^2 in col 192) + one sqrt ----
            d2r = sb.tile([128, 193], F32, tag="d2r")
            nc.vector.tensor_scalar(
                d2r[:, 0:192], t2_ps, 2.0 + DELTA, 0.0, op0=ALU.add,
                op1=ALU.max)
            nc.vector.tensor_scalar(
                d2r[:, 192:193], tp_ps, 2.0 + DELTA, 0.0, op0=ALU.add,
                op1=ALU.max)
            dms = sb.tile([128, 193], F32, tag="dms")
            nc.scalar.sqrt(dms, d2r)
            dposm = sb.tile([128, 1], F32, tag="dposm")
            nc.vector.tensor_scalar_add(dposm, dms[:, 192:193], MARGIN)

            # ---- weighted terms; sum on DVE, sign-count on ACT ----
            lwpre = sb.tile([128, 192], F32, tag="lwpre")
            nc.vector.scalar_tensor_tensor(
                lwpre, dms[:, 0:192], dposm, pm, op0=ALU.subtract,
                op1=ALU.mult)
            stacked = sb.tile([128, 2], F32, tag="stacked")
            lwj = sb.tile([128, 192], F32, tag="lwj")
            nc.vector.tensor_scalar(
                lwj, lwpre, 0.0, 0.0, op0=ALU.max, op1=ALU.add,
                accum_out=stacked[:, 0:1])
            sgj = sb.tile([128, 192], F32, tag="sgj")
            nc.scalar.activation(sgj, lwpre, ACT.Sign, bias=beps, scale=1.0,
                                 accum_out=stacked[:, 1:2])

            # ---- cross-partition reduce + writeback ----
            outp = ps.tile([1, 2], F32, tag="outp")
            nc.tensor.matmul(outp, onesc, stacked, start=True, stop=True)
            outs = sb.tile([1, 2], F32, tag="outs")
            nc.vector.tensor_copy(outs, outp)
            nc.sync.dma_start(out=out[:, :], in_=outs)

    nc.finalize()
    return nc


_NC_CACHE: dict = {}


def _get_nc() -> bass.Bass:
    if "nc" not in _NC_CACHE:
        _NC_CACHE["nc"] = build_nc()
    return _NC_CACHE["nc"]


def _make_consts() -> np.ndarray:
    cst = np.zeros((128, 448), dtype=np.float32)
    cst[:, 0:128] = np.eye(128)
    sel = np.zeros((48, 128), dtype=np.float32)
    p = np.arange(128)
    for c in range(48):
        sel[c, p % 64 == c] = 1.0
    cst[0:48, 128:256] = sel
    cst[0:48, 256:384] = -2.0 * sel
    cst[0, 384:448] = 1.0
    return cst.astype(ml_dtypes.bfloat16)


_CST = _make_consts()


def make_in_maps(output1, output2, weight):
    o1 = np.asarray(output1, dtype=np.float32)
    o2 = np.asarray(output2, dtype=np.float32)
    w = np.asarray(weight, dtype=np.float32)

    emb = np.concatenate([o1, o2], axis=0)
    w2 = np.tile(w, (2, 2))
    f8 = ml_dtypes.float8_e4m3
    a48 = np.arange(S)

    in_maps = []
    for c in range(NCORES):
        anchors = np.arange(c * S, c * S + S)
        pos = (anchors + B) % N
        used = np.zeros(N, dtype=bool)
        used[anchors] = True
        used[pos] = True
        loc = np.concatenate([anchors, pos, np.nonzero(~used)[0]])

        emb_loc = np.ascontiguousarray(emb[loc])
        embt = emb_loc.T
        ET = np.concatenate([embt[128 * k:128 * (k + 1), :] for k in range(4)],
                            axis=1).astype(f8)
        ER = np.concatenate([emb_loc[128 * t:128 * (t + 1), :] for t in range(3)],
                            axis=1).astype(f8)

        pmn = np.zeros((128, 192), dtype=np.float32)
        pmn[0:48, :] = -w2[anchors[:, None], loc[None, 0:192]]
        pmn[64:112, :] = -w2[anchors[:, None], loc[None, 192:384]]
        pmn[a48, a48] = 0.0          # k == i
        pmn[a48, S + a48] = 0.0      # k == p(i)

        in_maps.append({
            "er": ER,
            "et0": np.ascontiguousarray(ET[:, 0:768]),
            "et1": np.ascontiguousarray(ET[:, 768:1536]),
            "pmw": pmn.astype(ml_dtypes.bfloat16),
            "cst": _CST,
        })
    return in_maps


def reduce_outputs(results):
    parts = np.stack([np.asarray(r["out"][0], dtype=np.float64)
                      for r in results])
    total = parts.sum(axis=0)
    count = (total[1] + CELLS) / 2.0
    return np.asarray(
        np.float32(total[0]) / (np.float32(count) + np.float32(EPS)),
        dtype=np.float32)


def kernel(output1, output2, weight):
    in_maps = make_in_maps(output1, output2, weight)
    res = run_bass_kernel_spmd(_get_nc(), in_maps, core_ids=list(range(NCORES)))
    return reduce_outputs(res.results)
